# revision 1
# baseline (speedup 1.0000x reference)
"""KVCache prefill_draft eviction kernel for Trainium2 (8 NeuronCores).

Semantics (from the reference):
  - gather paged cache [1024,2,16,16,128] into per-seq linear [B=4, 2, L=4096, H=16, D=128]
  - sliding-window shift: new position l takes old position l+528 (for l<3568),
    new tokens k/v (for 3568<=l<4080), old position l (for l>=4080)
  - keys re-RoPE'd at position l (Neox rotate-half), values copied
  - scatter back to paged layout

528 = 33 pages, 3568 = 223 pages, 4080 = 255 pages -> everything is page-aligned.

Sharding: core c = 2*b + g handles sequence b, heads [8g, 8g+8).
Per-core inputs are host-sliced contiguous arrays; cos/sin tables replicated.
"""

import os
import numpy as np

KV_LEN = 4096
PAGE_SIZE = 16
SHIFT = 16
BSZ = 4
SEQ_LEN = 512
HEADS = 16
HEAD_DIM = 128
PAGES = 1024
PAGES_PER_SEQ = 256
ROPE_THETA = 10000.0

N_CORES = 8
HG = HEADS // 2            # heads per core = 8
PAGE_SHIFT = (SHIFT + SEQ_LEN) // PAGE_SIZE   # 33
OLD_PAGES = PAGES_PER_SEQ - PAGE_SHIFT        # 223 pages read from old cache
TOK_PAGE0 = OLD_PAGES                         # first out page fed by new tokens (223)
NEW_PAGES = SEQ_LEN // PAGE_SIZE              # 32 pages of new tokens
FREE = HG * HEAD_DIM                          # 1024 free elems per position
ROWBYTES = FREE * 4                           # 4KB

_CACHE = {}


def _rope_tables():
    """cos/sin tables in SBUF-resident layout [128, 4096]:
    tab[r, 128*t + j] = table[128*t + r, j] for position-tile t.
    Matches the reference's float32 angle computation."""
    try:
        import jax
        import jax.numpy as jnp

        cpu = jax.devices("cpu")[0]
        with jax.default_device(cpu):
            d = HEAD_DIM
            inv_freq = 1.0 / (
                ROPE_THETA ** (jnp.arange(0, d, 2, dtype=jnp.float32) / d)
            )
            pos = jnp.arange(KV_LEN, dtype=jnp.int32)
            ang = pos.astype(jnp.float32)[:, None] * inv_freq[None, :]
            cos_h = np.asarray(jnp.cos(ang), dtype=np.float32)
            sin_h = np.asarray(jnp.sin(ang), dtype=np.float32)
    except Exception:
        d = HEAD_DIM
        inv_freq = (
            np.float32(1.0)
            / (np.float32(ROPE_THETA) ** (np.arange(0, d, 2, dtype=np.float32) / np.float32(d)))
        ).astype(np.float32)
        ang = (np.arange(KV_LEN, dtype=np.float32)[:, None] * inv_freq[None, :]).astype(
            np.float32
        )
        a64 = ang.astype(np.float64)
        cos_h = np.cos(a64).astype(np.float32)
        sin_h = np.sin(a64).astype(np.float32)

    cos_full = np.concatenate([cos_h, cos_h], axis=1)          # [4096,128]
    sin_signed = np.concatenate([-sin_h, sin_h], axis=1)       # [4096,128]

    def to_sbuf_layout(tab):
        return np.ascontiguousarray(
            tab.reshape(KV_LEN // 128, 128, 128).transpose(1, 0, 2).reshape(128, KV_LEN)
        )

    return to_sbuf_layout(cos_full), to_sbuf_layout(sin_signed)


def _build_program():
    from contextlib import ExitStack

    import concourse.bacc as bacc
    import concourse.tile as tile
    import concourse.mybir as mybir

    f32 = mybir.dt.float32
    nc = bacc.Bacc(
        "TRN2", target_bir_lowering=False, debug=False, enable_asserts=False
    )

    old = nc.dram_tensor("old", [OLD_PAGES, 2, PAGE_SIZE, HG, HEAD_DIM], f32, kind="ExternalInput").ap()
    newkv = nc.dram_tensor("newkv", [2, SEQ_LEN, HG, HEAD_DIM], f32, kind="ExternalInput").ap()
    cos_d = nc.dram_tensor("cos_t", [128, KV_LEN], f32, kind="ExternalInput").ap()
    sin_d = nc.dram_tensor("sin_t", [128, KV_LEN], f32, kind="ExternalInput").ap()
    out = nc.dram_tensor("out", [PAGES_PER_SEQ, 2, PAGE_SIZE, HG, HEAD_DIM], f32, kind="ExternalOutput").ap()

    NT = KV_LEN // 128  # 32 position tiles

    with tile.TileContext(nc) as tc:
        with ExitStack() as ctx:
            tabs = ctx.enter_context(tc.tile_pool(name="tables", bufs=1))
            cos_sb = tabs.tile([128, KV_LEN], f32)
            sin_sb = tabs.tile([128, KV_LEN], f32)
            nc.sync.dma_start(cos_sb[:], cos_d)
            nc.sync.dma_start(sin_sb[:], sin_d)

            xp = ctx.enter_context(tc.tile_pool(name="x", bufs=4))
            t1p = ctx.enter_context(tc.tile_pool(name="t1", bufs=3))
            t2p = ctx.enter_context(tc.tile_pool(name="t2", bufs=3))
            outp = ctx.enter_context(tc.tile_pool(name="o", bufs=4))

            # v path: pure page-aligned copies, DRAM->DRAM, interleaved with the
            # k loop below so the ACT HWDGE ring drains smoothly.
            # chunks of OLD_PAGES split across NT iterations
            v_chunks = []
            base = 0
            for i in range(NT):
                n = (OLD_PAGES - base + (NT - i) - 1) // (NT - i)
                v_chunks.append((base, n))
                base += n
            assert base == OLD_PAGES

            for t in range(NT):
                X = xp.tile([128, FREE], f32)
                p0 = 8 * t
                if t * 128 + 127 < TOK_PAGE0 * PAGE_SIZE:
                    # whole tile from old pages (out pages p0..p0+7 <- old idx p0..p0+7)
                    nc.sync.dma_start(X[:], old[p0 : p0 + 8, 0])
                elif t == 27:
                    # pages 216..222 old (7 pages, 112 rows) + page 223 = tokens 0..15
                    nc.sync.dma_start(X[0:112, :], old[216:223, 0])
                    nc.sync.dma_start(X[112:128, :], newkv[0, 0:16])
                elif t <= 30:
                    tok = 16 + 128 * (t - 28)
                    nc.sync.dma_start(X[:], newkv[0, tok : tok + 128])
                else:
                    # tokens 400..511 (112 rows) + old page 255 (idx 222, 16 rows)
                    nc.sync.dma_start(X[0:112, :], newkv[0, 400:512])
                    nc.sync.dma_start(X[112:128, :], old[222, 0])

                Xr = X[:].rearrange("p (h d) -> p h d", d=HEAD_DIM)
                cos_b = (
                    cos_sb[:, 128 * t : 128 * (t + 1)]
                    .unsqueeze(1)
                    .broadcast_to([128, HG, HEAD_DIM])
                )
                sin_a = (
                    sin_sb[:, 128 * t : 128 * t + 64]
                    .unsqueeze(1)
                    .broadcast_to([128, HG, 64])
                )
                sin_b = (
                    sin_sb[:, 128 * t + 64 : 128 * t + 128]
                    .unsqueeze(1)
                    .broadcast_to([128, HG, 64])
                )

                T1 = t1p.tile([128, FREE], f32)
                T2 = t2p.tile([128, FREE], f32)
                T1r = T1[:].rearrange("p (h d) -> p h d", d=HEAD_DIM)
                T2r = T2[:].rearrange("p (h d) -> p h d", d=HEAD_DIM)

                nc.vector.tensor_mul(T1r, Xr, cos_b)
                nc.vector.tensor_mul(T2r[:, :, 0:64], Xr[:, :, 64:128], sin_a)
                nc.vector.tensor_mul(T2r[:, :, 64:128], Xr[:, :, 0:64], sin_b)

                O = outp.tile([128, FREE], f32)
                nc.vector.tensor_add(O[:], T1[:], T2[:])
                nc.scalar.dma_start(out[p0 : p0 + 8, 0], O[:])

                vb, vn = v_chunks[t]
                nc.scalar.dma_start(out[vb : vb + vn, 1], old[vb : vb + vn, 1])

            # new v tokens -> out pages 223..254
            nc.scalar.dma_start(out[TOK_PAGE0 : TOK_PAGE0 + NEW_PAGES, 1], newkv[1])
            # old page 255 v (old idx 222) -> out page 255
            nc.scalar.dma_start(out[255, 1], old[222, 1])

    nc.compile()
    return nc


def _get_program():
    if "nc" not in _CACHE:
        _CACHE["nc"] = _build_program()
    return _CACHE["nc"]


def kernel(
    k,
    v,
    draft_cache,
    kv_page_indices,
    bsz=BSZ,
    context_len=KV_LEN,
    seq_len=SEQ_LEN,
    n_local_heads=HEADS,
    head_dim=HEAD_DIM,
):
    from concourse.bass_utils import run_bass_kernel_spmd

    k = np.asarray(k, dtype=np.float32)
    v = np.asarray(v, dtype=np.float32)
    draft_cache = np.asarray(draft_cache, dtype=np.float32)
    kv_page_indices = np.asarray(kv_page_indices)

    pages = kv_page_indices.reshape(BSZ, PAGES_PER_SEQ)
    identity = bool(np.array_equal(kv_page_indices, np.arange(PAGES, dtype=kv_page_indices.dtype)))

    kb = k.reshape(BSZ, SEQ_LEN, HEADS, HEAD_DIM)
    vb = v.reshape(BSZ, SEQ_LEN, HEADS, HEAD_DIM)

    cos_pre, sin_pre = _CACHE.get("tables", (None, None))
    if cos_pre is None:
        cos_pre, sin_pre = _rope_tables()
        _CACHE["tables"] = (cos_pre, sin_pre)

    in_maps = []
    for c in range(N_CORES):
        b, g = divmod(c, 2)
        if identity:
            cache_b = draft_cache[b * PAGES_PER_SEQ : (b + 1) * PAGES_PER_SEQ]
        else:
            cache_b = draft_cache[pages[b]]
        old_c = np.ascontiguousarray(
            cache_b[PAGE_SHIFT:, :, :, g * HG : (g + 1) * HG, :]
        )
        newkv_c = np.stack(
            [
                kb[b, :, g * HG : (g + 1) * HG, :],
                vb[b, :, g * HG : (g + 1) * HG, :],
            ]
        ).astype(np.float32, copy=False)
        newkv_c = np.ascontiguousarray(newkv_c)
        in_maps.append(
            {
                "old": old_c,
                "newkv": newkv_c,
                "cos_t": cos_pre,
                "sin_t": sin_pre,
            }
        )

    nc = _get_program()
    trace = bool(int(os.environ.get("KVC_TRACE", "0")))
    res = run_bass_kernel_spmd(
        nc, in_maps, list(range(N_CORES)), trace=trace,
        trace_cores=list(range(N_CORES)) if trace else None,
    )
    _CACHE["last_results"] = res

    kv_pages = np.empty((PAGES, 2, PAGE_SIZE, HEADS, HEAD_DIM), dtype=np.float32)
    for c in range(N_CORES):
        b, g = divmod(c, 2)
        kv_pages[
            b * PAGES_PER_SEQ : (b + 1) * PAGES_PER_SEQ, :, :, g * HG : (g + 1) * HG, :
        ] = res.results[c]["out"]

    if identity:
        return kv_pages
    rotated = draft_cache.copy()
    rotated[kv_page_indices] = kv_pages
    return rotated


# revision 2
# speedup vs baseline: 1.4130x; 1.4130x over previous
"""KVCache prefill_draft eviction kernel for Trainium2 (8 NeuronCores).

Semantics (from the reference):
  - gather paged cache [1024,2,16,16,128] into per-seq linear [B=4, 2, L=4096, H=16, D=128]
  - sliding-window shift: new position l takes old position l+528 (l<3568),
    new tokens (3568<=l<4080), old position l (l>=4080)
  - keys re-RoPE'd at position l (Neox rotate-half), values copied
  - scatter back to paged layout

Sharding: core c = 2*b + g handles sequence b, heads [8g, 8g+8).

The host does all layout work so the device sees DMA-friendly shapes:
  - k source is pre-gathered (shift applied) and permuted to [128, 32*1024]:
    row r, block t = out position 128t+r, 8 heads x 128 dim. Every DMA row is
    16KB contiguous -> descriptors spread over all 16 SDMA engines.
  - v is a pure copy: contiguous old-page span + new tokens, moved DRAM->DRAM.
  - cos/sin tables [128, 4096] in the same position-permuted layout.
Device: 8 x [128,4096] k tiles -> RoPE (3 tensor_mul + 1 tensor_add on DVE,
tables broadcast over heads via stride-0 APs) -> store; v handled by gpsimd
DRAM->DRAM copies on an independent queue.
"""

import os
import numpy as np

KV_LEN = 4096
PAGE_SIZE = 16
SHIFT = 16
BSZ = 4
SEQ_LEN = 512
HEADS = 16
HEAD_DIM = 128
PAGES = 1024
PAGES_PER_SEQ = 256
ROPE_THETA = 10000.0

N_CORES = 8
HG = HEADS // 2                               # heads per core = 8
POS_SHIFT = SHIFT + SEQ_LEN                   # 528
KEEP = KV_LEN - POS_SHIFT                     # 3568 positions from old cache
TOK_END = KV_LEN - SHIFT                      # 4080
PAGE_SHIFT = POS_SHIFT // PAGE_SIZE           # 33
OLD_PAGES = PAGES_PER_SEQ - PAGE_SHIFT        # 223
FREE = HG * HEAD_DIM                          # 1024
NT = KV_LEN // 128                            # 32 position tiles
KW = NT * FREE                                # 32768 = k plane width per row
TILE_W = 4096                                 # free width per worked tile
NTILES = KW // TILE_W                         # 8
SUBT = TILE_W // FREE                         # 4 position-tiles per worked tile

_CACHE = {}


def _rope_tables():
    """cos/sin tables [128, 4096] in position-permuted layout:
    tab[r, 128*t + j] = table[128*t + r, j]. Matches the reference's
    float32 angle computation (f32 pow/mul, then cos/sin)."""
    try:
        import jax
        import jax.numpy as jnp

        cpu = jax.devices("cpu")[0]
        with jax.default_device(cpu):
            d = HEAD_DIM
            inv_freq = 1.0 / (
                ROPE_THETA ** (jnp.arange(0, d, 2, dtype=jnp.float32) / d)
            )
            pos = jnp.arange(KV_LEN, dtype=jnp.int32)
            ang = pos.astype(jnp.float32)[:, None] * inv_freq[None, :]
            cos_h = np.asarray(jnp.cos(ang), dtype=np.float32)
            sin_h = np.asarray(jnp.sin(ang), dtype=np.float32)
    except Exception:
        d = HEAD_DIM
        inv_freq = (
            np.float32(1.0)
            / (np.float32(ROPE_THETA) ** (np.arange(0, d, 2, dtype=np.float32) / np.float32(d)))
        ).astype(np.float32)
        ang = (np.arange(KV_LEN, dtype=np.float32)[:, None] * inv_freq[None, :]).astype(
            np.float32
        )
        a64 = ang.astype(np.float64)
        cos_h = np.cos(a64).astype(np.float32)
        sin_h = np.sin(a64).astype(np.float32)

    cos_full = np.concatenate([cos_h, cos_h], axis=1)          # [4096,128]
    sin_signed = np.concatenate([-sin_h, sin_h], axis=1)       # [4096,128]

    def perm(tab):
        return np.ascontiguousarray(
            tab.reshape(NT, 128, 128).transpose(1, 0, 2).reshape(128, KV_LEN)
        )

    return perm(cos_full), perm(sin_signed)


def _build_program():
    from contextlib import ExitStack

    import concourse.bacc as bacc
    import concourse.tile as tile
    import concourse.mybir as mybir

    f32 = mybir.dt.float32
    nc = bacc.Bacc(
        "TRN2", target_bir_lowering=False, debug=False, enable_asserts=False
    )

    srck = nc.dram_tensor("srck", [128, KW], f32, kind="ExternalInput").ap()
    oldv = nc.dram_tensor("oldv", [OLD_PAGES, PAGE_SIZE, HG, HEAD_DIM], f32, kind="ExternalInput").ap()
    newv = nc.dram_tensor("newv", [SEQ_LEN, HG, HEAD_DIM], f32, kind="ExternalInput").ap()
    cos_d = nc.dram_tensor("cos_t", [128, KV_LEN], f32, kind="ExternalInput").ap()
    sin_d = nc.dram_tensor("sin_t", [128, KV_LEN], f32, kind="ExternalInput").ap()
    outk = nc.dram_tensor("out_k", [128, KW], f32, kind="ExternalOutput").ap()
    outv = nc.dram_tensor("out_v", [PAGES_PER_SEQ, PAGE_SIZE, HG, HEAD_DIM], f32, kind="ExternalOutput").ap()

    with tile.TileContext(nc) as tc:
        with ExitStack() as ctx:
            tabs = ctx.enter_context(tc.tile_pool(name="tables", bufs=1))
            cos_sb = tabs.tile([128, KV_LEN], f32)
            sin_sb = tabs.tile([128, KV_LEN], f32)
            nc.sync.dma_start(cos_sb[:], cos_d)
            nc.sync.dma_start(sin_sb[:], sin_d)

            # v path: contiguous DRAM->DRAM copies on the gpsimd (SWDGE)
            # queue, independent of both HWDGE rings.
            # out pages 0..222 <- old pages 33..255 (= oldv[0:223])
            nc.gpsimd.dma_start(
                outv[0:OLD_PAGES], oldv[:], max_dma_last_dim=16384
            )
            # out pages 223..254 <- new v tokens
            nc.gpsimd.dma_start(
                outv[OLD_PAGES : OLD_PAGES + SEQ_LEN // PAGE_SIZE],
                newv[:],
                max_dma_last_dim=16384,
            )
            # out page 255 <- old page 255 (= oldv[222])
            nc.gpsimd.dma_start(outv[255], oldv[OLD_PAGES - 1])

            xp = ctx.enter_context(tc.tile_pool(name="x", bufs=3))
            t1p = ctx.enter_context(tc.tile_pool(name="t1", bufs=1))
            t2p = ctx.enter_context(tc.tile_pool(name="t2", bufs=1))
            outp = ctx.enter_context(tc.tile_pool(name="o", bufs=3))

            for i in range(NTILES):
                X = xp.tile([128, TILE_W], f32)
                nc.sync.dma_start(X[:], srck[:, i * TILE_W : (i + 1) * TILE_W])

                Xr = X[:].rearrange("p (s h d) -> p s h d", h=HG, d=HEAD_DIM)
                cs = cos_sb[:, i * SUBT * 128 : (i + 1) * SUBT * 128].rearrange(
                    "p (s j) -> p s j", j=128
                )
                sn = sin_sb[:, i * SUBT * 128 : (i + 1) * SUBT * 128].rearrange(
                    "p (s j) -> p s j", j=128
                )
                cos_b = cs.unsqueeze(2).broadcast_to([128, SUBT, HG, 128])
                sin_a = sn[:, :, 0:64].unsqueeze(2).broadcast_to([128, SUBT, HG, 64])
                sin_b = sn[:, :, 64:128].unsqueeze(2).broadcast_to([128, SUBT, HG, 64])

                T1 = t1p.tile([128, TILE_W], f32)
                T2 = t2p.tile([128, TILE_W], f32)
                T1r = T1[:].rearrange("p (s h d) -> p s h d", h=HG, d=HEAD_DIM)
                T2r = T2[:].rearrange("p (s h d) -> p s h d", h=HG, d=HEAD_DIM)

                nc.vector.tensor_mul(T1r, Xr, cos_b)
                nc.vector.tensor_mul(T2r[:, :, :, 0:64], Xr[:, :, :, 64:128], sin_a)
                nc.vector.tensor_mul(T2r[:, :, :, 64:128], Xr[:, :, :, 0:64], sin_b)

                O = outp.tile([128, TILE_W], f32)
                nc.vector.tensor_add(O[:], T1[:], T2[:])
                nc.scalar.dma_start(outk[:, i * TILE_W : (i + 1) * TILE_W], O[:])

    nc.compile()
    return nc


def _get_program():
    if "nc" not in _CACHE:
        _CACHE["nc"] = _build_program()
    return _CACHE["nc"]


def kernel(
    k,
    v,
    draft_cache,
    kv_page_indices,
    bsz=BSZ,
    context_len=KV_LEN,
    seq_len=SEQ_LEN,
    n_local_heads=HEADS,
    head_dim=HEAD_DIM,
):
    from concourse.bass_utils import run_bass_kernel_spmd

    k = np.asarray(k, dtype=np.float32)
    v = np.asarray(v, dtype=np.float32)
    draft_cache = np.asarray(draft_cache, dtype=np.float32)
    kv_page_indices = np.asarray(kv_page_indices)

    pages = kv_page_indices.reshape(BSZ, PAGES_PER_SEQ)
    identity = bool(
        np.array_equal(kv_page_indices, np.arange(PAGES, dtype=kv_page_indices.dtype))
    )

    kb = k.reshape(BSZ, SEQ_LEN, HEADS, HEAD_DIM)
    vb = v.reshape(BSZ, SEQ_LEN, HEADS, HEAD_DIM)

    if "tables" not in _CACHE:
        _CACHE["tables"] = _rope_tables()
    cos_pre, sin_pre = _CACHE["tables"]

    in_maps = []
    for c in range(N_CORES):
        b, g = divmod(c, 2)
        if identity:
            cache_b = draft_cache[b * PAGES_PER_SEQ : (b + 1) * PAGES_PER_SEQ]
        else:
            cache_b = draft_cache[pages[b]]
        hsl = slice(g * HG, (g + 1) * HG)

        # k source with the shift applied, then position-permuted
        oldk = cache_b[:, 0, :, hsl, :].reshape(KV_LEN, HG, HEAD_DIM)
        srck = np.empty((KV_LEN, HG, HEAD_DIM), np.float32)
        srck[0:KEEP] = oldk[POS_SHIFT:KV_LEN]
        srck[KEEP:TOK_END] = kb[b, :, hsl, :]
        srck[TOK_END:] = oldk[TOK_END:]
        srck_perm = np.ascontiguousarray(
            srck.reshape(NT, 128, FREE).transpose(1, 0, 2).reshape(128, KW)
        )

        oldv_c = np.ascontiguousarray(cache_b[PAGE_SHIFT:, 1, :, hsl, :])
        newv_c = np.ascontiguousarray(vb[b, :, hsl, :])

        in_maps.append(
            {
                "srck": srck_perm,
                "oldv": oldv_c,
                "newv": newv_c,
                "cos_t": cos_pre,
                "sin_t": sin_pre,
            }
        )

    nc = _get_program()
    trace = bool(int(os.environ.get("KVC_TRACE", "0")))
    res = run_bass_kernel_spmd(
        nc,
        in_maps,
        list(range(N_CORES)),
        trace=trace,
        trace_cores=list(range(N_CORES)) if trace else None,
    )
    _CACHE["last_results"] = res

    kv_pages = np.empty((PAGES, 2, PAGE_SIZE, HEADS, HEAD_DIM), dtype=np.float32)
    for c in range(N_CORES):
        b, g = divmod(c, 2)
        hsl = slice(g * HG, (g + 1) * HG)
        psl = slice(b * PAGES_PER_SEQ, (b + 1) * PAGES_PER_SEQ)
        outk = res.results[c]["out_k"]
        klin = (
            outk.reshape(128, NT, HG, HEAD_DIM)
            .transpose(1, 0, 2, 3)
            .reshape(PAGES_PER_SEQ, PAGE_SIZE, HG, HEAD_DIM)
        )
        kv_pages[psl, 0, :, hsl, :] = klin
        kv_pages[psl, 1, :, hsl, :] = res.results[c]["out_v"]

    if identity:
        return kv_pages
    rotated = draft_cache.copy()
    rotated[kv_page_indices] = kv_pages
    return rotated
